# revision 14
# baseline (speedup 1.0000x reference)
"""Trainium2 Bass kernel for nn_NodeAttentionPerMetaPath (GAT-style node attention).

Reference computation (N=8192, F_IN=256, d=64):
    h      = x @ trans                      # [N, d]
    e1     = h @ attn[:d];  e2 = h @ attn[d:]
    scores = leaky_relu(e1 + e2.T, 0.2)     # [N, N]
    masked = where(mask==0, -1e15, scores)
    out    = softmax(masked, axis=1) @ h    # [N, d]

Sharding: rows r across 8 cores (1024 each); every core computes the full
h/e locally from a replicated fp16 x (no collectives at all).

Algebra (exp monotone, alpha<1):
    exp(leaky(e1+e2)) / exp(a*e1) = B2[j] * max(C[r]*D[j], 1)
    C = exp(.8 e1), D = exp(.8 e2), B2 = exp(.2 e2), C*D*B2 = C*exp(e2)
    out = (P @ h) / (P @ 1),  P = mask * B2 * max(CD, 1)
        = mask * max(C_rep * db, b2)        (one fused 4x tensor_scalar)

Layout: scores live TRANSPOSED [j-part, r-free] from the start.  The mask
is packed to fp16 on the host and transpose-loaded straight from DRAM by
the DMA XBAR (dma_start(transpose=True)), so the [N,N] work needs NO PE
transposes and NO PSUM->SBUF copies: per 128-j chunk it is one TS, one
masked multiply (split DVE/GPSIMD), and one accumulated matmul whose
ones-column yields the softmax denominator for free.
"""

from contextlib import ExitStack

import numpy as np

import concourse.bass as bass
import concourse.bacc as bacc
import concourse.mybir as mybir
import concourse.tile as tile
from concourse.bass_utils import run_bass_kernel_spmd
from concourse.masks import make_identity

f32 = mybir.dt.float32
fp16 = mybir.dt.float16

Exp = mybir.ActivationFunctionType.Exp

N_CORES = 8
N = 8192
F_IN = 256
D = 64
ALPHA = 0.2

R = N // N_CORES   # rows per core
JC = N // 128      # j-chunks
NG = 16            # mask xbar groups
CG = JC // NG      # chunks per group (4)
HQ = 16            # h-compute groups (4 chunks each)

# masked-multiply split: columns [0:TT_SPLIT] on DVE, rest on GPSIMD
TT_SPLIT = 592


def build_kernel(ctx: ExitStack, tc: tile.TileContext, mask16_c, x_h, trans_h, transT_h, a12h, outT):
    nc = tc.nc

    singles = ctx.enter_context(tc.tile_pool(name="singles", bufs=1))
    hps_pool = ctx.enter_context(tc.tile_pool(name="hps", bufs=2, space="PSUM"))
    ps_o = ctx.enter_context(tc.tile_pool(name="ps_o", bufs=1, space="PSUM"))
    ps_r = ctx.enter_context(tc.tile_pool(name="ps_r", bufs=1, space="PSUM"))
    work = ctx.enter_context(tc.tile_pool(name="work", bufs=3))
    outp = ctx.enter_context(tc.tile_pool(name="outp", bufs=2))
    dram = ctx.enter_context(tc.tile_pool(name="dram", bufs=1, space="DRAM"))

    # ---- DMA streams.  ALL XBAR transposes share one HWDGE queue (sync):
    # concurrent XBAR streams on two queues corrupt each other.
    # x first (it unblocks all compute), then the mask.
    trans_w = singles.tile([128, 2, D + 2], fp16)
    nc.scalar.dma_start(
        out=trans_w[:, :, 0:D], in_=trans_h.rearrange("(c p) d -> p c d", p=128)
    )
    transT_sb = singles.tile([D, F_IN], fp16)
    nc.scalar.dma_start(out=transT_sb, in_=transT_h[:, :])
    a12_sb = singles.tile([D, 2], fp16)
    nc.scalar.dma_start(out=a12_sb, in_=a12h[:, :])
    # xT[p, fc, j] = x[j, fc*128+p]; two tiles so each XBAR dest is contiguous
    xT = [singles.tile([128, 2, N // 2], fp16, tag=f"xT{i}", name=f"xT{i}") for i in range(2)]
    for i in range(2):
        nc.sync.dma_start(
            out=xT[i], in_=x_h[i * (N // 2) : (i + 1) * (N // 2), :], transpose=True
        )

    # mask transpose-stream: mt_g[p, k, r] = mask[r, (g*CG+k)*128+p]
    mts = []
    for g in range(NG):
        mt = singles.tile([128, CG, R], fp16, tag=f"mt{g}", name=f"mt{g}")
        cols = slice(g * CG * 128, (g + 1) * CG * 128)
        nc.sync.dma_start(out=mt, in_=mask16_c[:, cols], transpose=True)
        mts.append(mt)

    ident = singles.tile([128, 128], fp16)
    make_identity(nc, ident)
    ones_row_f = singles.tile([1, D], f32)
    nc.vector.memset(ones_row_f, 1.0)

    # ---- w12 = trans.T @ a12  -> moving-operand columns 64:66
    for fc in range(2):
        wps = hps_pool.tile([128, 2], f32, tag="wps", bufs=1)
        nc.tensor.matmul(
            wps, transT_sb[:, fc * 128 : (fc + 1) * 128], a12_sb, start=True, stop=True
        )
        nc.vector.tensor_copy(trans_w[:, fc, D : D + 2], wps)

    # ---- full h/e per 4-chunk groups.  Inputs are rolled per core so the
    # own 1024 rows are always chunks 0..7 (groups 0 and 1).
    h_sb = singles.tile([128, JC, D + 1], fp16)   # [j%128, jc, h | 1]
    nc.vector.memset(h_sb[:, :, D], 1.0)
    c_all = singles.tile([128, JC], fp16)   # exp(.8 e1)
    db_all = singles.tile([128, JC], f32)  # exp(e2)
    b2_all = singles.tile([128, JC], f32)  # exp(.2 e2)
    C_rep = singles.tile([128, R], fp16)

    for q in range(HQ):
        hps = hps_pool.tile([128, 4, D + 2], f32, tag="hps")
        for k in range(4):
            jc = q * 4 + k
            xa = xT[jc // 32]
            j0 = (jc % 32) * 128
            for fc in range(2):
                nc.tensor.matmul(
                    hps[:, k, :],
                    xa[:, fc, j0 : j0 + 128],
                    trans_w[:, fc, :],
                    start=(fc == 0),
                    stop=(fc == 1),
                )
        cols = slice(q * 4, q * 4 + 4)
        nc.scalar.copy(h_sb[:, cols, 0:D], hps[:, :, 0:D])
        nc.scalar.activation(c_all[:, cols], hps[:, :, D], Exp, scale=1.0 - ALPHA)
        nc.scalar.activation(db_all[:, cols], hps[:, :, D + 1], Exp, scale=1.0)
        nc.scalar.activation(b2_all[:, cols], hps[:, :, D + 1], Exp, scale=ALPHA)

        if q == 1:
            # ---- C_rep: own-row C values replicated across partitions.
            # Transpose each own column of c_all to a [1, 128] row (PE),
            # assemble [1, R], then broadcast via K=1 matmuls.
            # (No DRAM roundtrip, no DMA in the middle of the XBAR stream.)
            c_row1 = singles.tile([1, R], fp16)
            crps = hps_pool.tile([1, 8, 128], fp16, tag="crps", bufs=1)
            for rb in range(8):
                nc.tensor.transpose(crps[:, rb, :], c_all[:, rb : rb + 1], ident)
            nc.vector.tensor_copy(c_row1, crps.rearrange("p a b -> p (a b)"))
            ones_col = singles.tile([1, 128], fp16)
            nc.vector.memset(ones_col, 1.0)
            for half in range(2):
                crp = hps_pool.tile([128, 512], f32, tag="crp", bufs=1)
                nc.tensor.matmul(
                    crp,
                    ones_col,
                    c_row1[:, half * 512 : (half + 1) * 512],
                    start=True,
                    stop=True,
                )
                nc.vector.tensor_copy(C_rep[:, half * 512 : (half + 1) * 512], crp)

    # ---- main loop: one TS + split TT + accumulated matmul per j-chunk
    po = [ps_o.tile([D + 1, 512], f32, tag=f"po{i}", name=f"po{i}") for i in range(2)]
    for jc in range(JC):
        mt = mts[jc // CG]
        k = jc % CG
        v = work.tile([128, R], fp16, tag="v")
        # v = max(C_rep * exp(e2[jc]), exp(.2 e2[jc]))  (= B2 * max(CD, 1))
        nc.vector.tensor_scalar(
            v,
            C_rep,
            db_all[:, jc : jc + 1],
            b2_all[:, jc : jc + 1],
            mybir.AluOpType.mult,
            mybir.AluOpType.max,
        )
        # P = mask * v, in place (split DVE / GPSIMD)
        nc.vector.tensor_tensor(
            v[:, 0:TT_SPLIT], v[:, 0:TT_SPLIT], mt[:, k, 0:TT_SPLIT], mybir.AluOpType.mult
        )
        nc.gpsimd.tensor_tensor(
            v[:, TT_SPLIT:], v[:, TT_SPLIT:], mt[:, k, TT_SPLIT:], mybir.AluOpType.mult
        )
        for i in range(2):
            nc.tensor.matmul(
                po[i],
                h_sb[:, jc, :],
                v[:, i * 512 : (i + 1) * 512],
                start=(jc == 0),
                stop=(jc == JC - 1),
            )

    # ---- normalize: out = numer * (1/denom), denom broadcast via K=1 outer
    for i in range(2):
        recip = outp.tile([1, 512], f32, tag="recip")
        nc.vector.reciprocal(recip, po[i][D : D + 1, :])
        rr = ps_r.tile([D, 512], f32, tag="rr")
        nc.tensor.matmul(rr, ones_row_f, recip, start=True, stop=True)
        rr_sb = outp.tile([D, 512], f32, tag="rr_sb")
        nc.vector.tensor_copy(rr_sb, rr)
        o_t = outp.tile([D, 512], f32, tag="o_t")
        nc.vector.tensor_tensor(o_t, po[i][0:D, :], rr_sb, mybir.AluOpType.mult)
        nc.gpsimd.dma_start(out=outT[:, i * 512 : (i + 1) * 512], in_=o_t)


def build_nc():
    nc = bacc.Bacc("TRN2", num_devices=N_CORES)
    mask16_c = nc.dram_tensor("mask16_c", [R, N], fp16, kind="ExternalInput")
    x_h = nc.dram_tensor("x_h", [N, F_IN], fp16, kind="ExternalInput")
    trans_h = nc.dram_tensor("trans_h", [F_IN, D], fp16, kind="ExternalInput")
    transT_h = nc.dram_tensor("transT_h", [D, F_IN], fp16, kind="ExternalInput")
    a12h = nc.dram_tensor("a12h", [D, 2], fp16, kind="ExternalInput")
    outT = nc.dram_tensor("outT", [D, R], f32, kind="ExternalOutput")
    with ExitStack() as ctx:
        tc = ctx.enter_context(tile.TileContext(nc))
        build_kernel(
            ctx, tc, mask16_c[:, :], x_h[:, :], trans_h[:, :],
            transT_h[:, :], a12h[:, :], outT[:, :],
        )
    nc.compile()
    return nc


LAST_RESULTS = None


def kernel(x, mask, trans, attn, _trace=False):
    x16 = np.ascontiguousarray(np.asarray(x), dtype=np.float16)
    mask16 = np.ascontiguousarray(np.asarray(mask), dtype=np.float16)
    trans16 = np.ascontiguousarray(np.asarray(trans), dtype=np.float16)
    transT16 = np.ascontiguousarray(np.asarray(trans).T, dtype=np.float16)
    attn = np.asarray(attn, dtype=np.float16)
    a12 = np.ascontiguousarray(np.concatenate([attn[:D], attn[D:]], axis=1))

    nc = build_nc()
    # identical SPMD program on every core: roll x rows / mask columns by
    # -c*R so each core's own rows are always j-chunks 0..7 (a column
    # permutation inside the softmax sum; the result is unchanged)
    in_maps = [
        {
            "mask16_c": np.ascontiguousarray(
                np.roll(mask16[c * R : (c + 1) * R], -c * R, axis=1)
            ),
            "x_h": np.ascontiguousarray(np.roll(x16, -c * R, axis=0)),
            "trans_h": trans16,
            "transT_h": transT16,
            "a12h": a12,
        }
        for c in range(N_CORES)
    ]
    res = run_bass_kernel_spmd(nc, in_maps, list(range(N_CORES)), trace=_trace)
    global LAST_RESULTS
    LAST_RESULTS = res
    out = np.concatenate(
        [res.results[c]["outT"].T for c in range(N_CORES)], axis=0
    )
    return np.ascontiguousarray(out, dtype=np.float32)


if __name__ == "__main__":
    nc = build_nc()
    print("built OK")


# revision 16
# speedup vs baseline: 1.1296x; 1.1296x over previous
"""Trainium2 Bass kernel for nn_NodeAttentionPerMetaPath (GAT-style node attention).

Reference computation (N=8192, F_IN=256, d=64):
    h      = x @ trans                      # [N, d]
    e1     = h @ attn[:d];  e2 = h @ attn[d:]
    scores = leaky_relu(e1 + e2.T, 0.2)     # [N, N]
    masked = where(mask==0, -1e15, scores)
    out    = softmax(masked, axis=1) @ h    # [N, d]

Sharding: rows r across 8 cores (1024 each); every core computes the full
h/e locally from a replicated fp16 x (no collectives at all).

Algebra (exp monotone, alpha<1):
    exp(leaky(e1+e2)) / exp(a*e1) = B2[j] * max(C[r]*D[j], 1)
    C = exp(.8 e1), D = exp(.8 e2), B2 = exp(.2 e2), C*D*B2 = C*exp(e2)
    out = (P @ h) / (P @ 1),  P = mask * B2 * max(CD, 1)
        = mask * max(C_rep * db, b2)        (one fused 4x tensor_scalar)

Layout: scores live TRANSPOSED [j-part, r-free] from the start.  The mask
is packed to fp16 on the host and transpose-loaded straight from DRAM by
the DMA XBAR (dma_start(transpose=True)), so the [N,N] work needs NO PE
transposes and NO PSUM->SBUF copies: per 128-j chunk it is one TS, one
masked multiply (split DVE/GPSIMD), and one accumulated matmul whose
ones-column yields the softmax denominator for free.
"""

from contextlib import ExitStack

import numpy as np

import concourse.bass as bass
import concourse.bacc as bacc
import concourse.mybir as mybir
import concourse.tile as tile
from concourse.bass_utils import run_bass_kernel_spmd
from concourse.masks import make_identity

f32 = mybir.dt.float32
fp16 = mybir.dt.float16

Exp = mybir.ActivationFunctionType.Exp

N_CORES = 8
N = 8192
F_IN = 256
D = 64
ALPHA = 0.2

R = N // N_CORES   # rows per core
JC = N // 128      # j-chunks
NG = 8             # mask xbar groups
CG = JC // NG      # chunks per group (8)
HQ = 16            # h-compute groups (4 chunks each)

# masked-multiply split per chunk-PAIR [128, 2048]:
# columns [0:TT_SPLIT] on DVE, rest on GPSIMD
TT_SPLIT = 1056


def build_kernel(ctx: ExitStack, tc: tile.TileContext, mask16_c, x_h, trans_h, transT_h, a12h, outT):
    nc = tc.nc

    singles = ctx.enter_context(tc.tile_pool(name="singles", bufs=1))
    hps_pool = ctx.enter_context(tc.tile_pool(name="hps", bufs=2, space="PSUM"))
    ps_o = ctx.enter_context(tc.tile_pool(name="ps_o", bufs=1, space="PSUM"))
    ps_r = ctx.enter_context(tc.tile_pool(name="ps_r", bufs=1, space="PSUM"))
    work = ctx.enter_context(tc.tile_pool(name="work", bufs=3))
    outp = ctx.enter_context(tc.tile_pool(name="outp", bufs=2))
    dram = ctx.enter_context(tc.tile_pool(name="dram", bufs=1, space="DRAM"))

    # ---- DMA streams.  ALL XBAR transposes share one HWDGE queue (sync):
    # concurrent XBAR streams on two queues corrupt each other.
    # x first (it unblocks all compute), then the mask.
    trans_w = singles.tile([128, 2, D + 2], fp16)
    nc.scalar.dma_start(
        out=trans_w[:, :, 0:D], in_=trans_h.rearrange("(c p) d -> p c d", p=128)
    )
    transT_sb = singles.tile([D, F_IN], fp16)
    nc.scalar.dma_start(out=transT_sb, in_=transT_h[:, :])
    a12_sb = singles.tile([D, 2], fp16)
    nc.scalar.dma_start(out=a12_sb, in_=a12h[:, :])
    # xT[p, fc, j] = x[j, fc*128+p]; two tiles so each XBAR dest is contiguous
    xT = [singles.tile([128, 2, N // 2], fp16, tag=f"xT{i}", name=f"xT{i}") for i in range(2)]
    for i in range(2):
        nc.sync.dma_start(
            out=xT[i], in_=x_h[i * (N // 2) : (i + 1) * (N // 2), :], transpose=True
        )

    # mask transpose-stream: mt_g[p, k, r] = mask[r, (g*CG+k)*128+p]
    mts = []
    for g in range(NG):
        mt = singles.tile([128, CG, R], fp16, tag=f"mt{g}", name=f"mt{g}")
        cols = slice(g * CG * 128, (g + 1) * CG * 128)
        nc.sync.dma_start(out=mt, in_=mask16_c[:, cols], transpose=True)
        mts.append(mt)

    ident = singles.tile([128, 128], fp16)
    make_identity(nc, ident)
    ones_row_f = singles.tile([1, D], f32)
    nc.vector.memset(ones_row_f, 1.0)

    # ---- w12 = trans.T @ a12  -> moving-operand columns 64:66
    for fc in range(2):
        wps = hps_pool.tile([128, 2], f32, tag="wps", bufs=1)
        nc.tensor.matmul(
            wps, transT_sb[:, fc * 128 : (fc + 1) * 128], a12_sb, start=True, stop=True
        )
        nc.vector.tensor_copy(trans_w[:, fc, D : D + 2], wps)

    # ---- full h/e per 4-chunk groups.  Inputs are rolled per core so the
    # own 1024 rows are always chunks 0..7 (groups 0 and 1).
    h_sb = singles.tile([128, JC, D + 1], fp16)   # [j%128, jc, h | 1]
    nc.vector.memset(h_sb[:, :, D], 1.0)
    c_all = singles.tile([128, JC], fp16)   # exp(.8 e1)
    db_all = singles.tile([128, JC], f32)  # exp(e2)
    b2_all = singles.tile([128, JC], f32)  # exp(.2 e2)
    C_rep = singles.tile([128, R], fp16)

    for q in range(HQ):
        hps = hps_pool.tile([128, 4, D + 2], f32, tag="hps")
        for k in range(4):
            jc = q * 4 + k
            xa = xT[jc // 32]
            j0 = (jc % 32) * 128
            for fc in range(2):
                nc.tensor.matmul(
                    hps[:, k, :],
                    xa[:, fc, j0 : j0 + 128],
                    trans_w[:, fc, :],
                    start=(fc == 0),
                    stop=(fc == 1),
                )
        cols = slice(q * 4, q * 4 + 4)
        nc.scalar.copy(h_sb[:, cols, 0:D], hps[:, :, 0:D])
        nc.scalar.activation(c_all[:, cols], hps[:, :, D], Exp, scale=1.0 - ALPHA)
        nc.scalar.activation(db_all[:, cols], hps[:, :, D + 1], Exp, scale=1.0)
        nc.scalar.activation(b2_all[:, cols], hps[:, :, D + 1], Exp, scale=ALPHA)

        if q == 1:
            # ---- C_rep: own-row C values replicated across partitions.
            # Transpose each own column of c_all to a [1, 128] row (PE),
            # assemble [1, R], then broadcast via K=1 matmuls.
            # (No DRAM roundtrip, no DMA in the middle of the XBAR stream.)
            c_row1 = singles.tile([1, R], fp16)
            crps = hps_pool.tile([1, 8, 128], fp16, tag="crps", bufs=1)
            for rb in range(8):
                nc.tensor.transpose(crps[:, rb, :], c_all[:, rb : rb + 1], ident)
            nc.vector.tensor_copy(c_row1, crps.rearrange("p a b -> p (a b)"))
            ones_col = singles.tile([1, 128], fp16)
            nc.vector.memset(ones_col, 1.0)
            for half in range(2):
                crp = hps_pool.tile([128, 512], f32, tag="crp", bufs=1)
                nc.tensor.matmul(
                    crp,
                    ones_col,
                    c_row1[:, half * 512 : (half + 1) * 512],
                    start=True,
                    stop=True,
                )
                nc.vector.tensor_copy(C_rep[:, half * 512 : (half + 1) * 512], crp)

    # ---- main loop: chunk-PAIRS.  Two 4x TS fills, one DVE TT + one
    # GPSIMD TT over the flattened [128, 2048], four accumulated matmuls.
    po = [ps_o.tile([D + 1, 512], f32, tag=f"po{i}", name=f"po{i}") for i in range(2)]
    M = mybir.AluOpType.mult
    for p in range(JC // 2):
        jc0 = 2 * p
        mt = mts[jc0 // CG]
        k = jc0 % CG
        vp = work.tile([128, 2, R], fp16, tag="vp")
        for half in range(2):
            jc = jc0 + half
            # v = max(C_rep * exp(e2[jc]), exp(.2 e2[jc]))  (= B2 * max(CD, 1))
            nc.vector.tensor_scalar(
                vp[:, half, :],
                C_rep,
                db_all[:, jc : jc + 1],
                b2_all[:, jc : jc + 1],
                M,
                mybir.AluOpType.max,
            )
        # P = mask * v, in place (split DVE / GPSIMD)
        vf = vp.rearrange("p a b -> p (a b)")
        mf = mt[:, k : k + 2, :].rearrange("p a b -> p (a b)")
        nc.vector.tensor_tensor(vf[:, 0:TT_SPLIT], vf[:, 0:TT_SPLIT], mf[:, 0:TT_SPLIT], M)
        nc.gpsimd.tensor_tensor(vf[:, TT_SPLIT:], vf[:, TT_SPLIT:], mf[:, TT_SPLIT:], M)
        for half in range(2):
            jc = jc0 + half
            for i in range(2):
                nc.tensor.matmul(
                    po[i],
                    h_sb[:, jc, :],
                    vp[:, half, i * 512 : (i + 1) * 512],
                    start=(jc == 0),
                    stop=(jc == JC - 1),
                )

    # ---- normalize: out = numer * (1/denom).  Broadcast the denominator
    # row via a K=1 outer product FIRST, then reciprocal on 64 lanes.
    for i in range(2):
        dcp = outp.tile([1, 512], f32, tag="dcp")
        nc.vector.tensor_copy(dcp, po[i][D : D + 1, :])
        rr = ps_r.tile([D, 512], f32, tag="rr")
        nc.tensor.matmul(rr, ones_row_f, dcp, start=True, stop=True)
        rr_sb = outp.tile([D, 512], f32, tag="rr_sb")
        nc.vector.reciprocal(rr_sb, rr)
        o_t = outp.tile([D, 512], f32, tag="o_t")
        nc.vector.tensor_tensor(o_t, po[i][0:D, :], rr_sb, M)
        nc.gpsimd.dma_start(out=outT[:, i * 512 : (i + 1) * 512], in_=o_t)


def build_nc():
    nc = bacc.Bacc("TRN2", num_devices=N_CORES)
    mask16_c = nc.dram_tensor("mask16_c", [R, N], fp16, kind="ExternalInput")
    x_h = nc.dram_tensor("x_h", [N, F_IN], fp16, kind="ExternalInput")
    trans_h = nc.dram_tensor("trans_h", [F_IN, D], fp16, kind="ExternalInput")
    transT_h = nc.dram_tensor("transT_h", [D, F_IN], fp16, kind="ExternalInput")
    a12h = nc.dram_tensor("a12h", [D, 2], fp16, kind="ExternalInput")
    outT = nc.dram_tensor("outT", [D, R], f32, kind="ExternalOutput")
    with ExitStack() as ctx:
        tc = ctx.enter_context(tile.TileContext(nc))
        build_kernel(
            ctx, tc, mask16_c[:, :], x_h[:, :], trans_h[:, :],
            transT_h[:, :], a12h[:, :], outT[:, :],
        )
    nc.compile()
    return nc


LAST_RESULTS = None


def kernel(x, mask, trans, attn, _trace=False):
    x16 = np.ascontiguousarray(np.asarray(x), dtype=np.float16)
    mask16 = np.ascontiguousarray(np.asarray(mask), dtype=np.float16)
    trans16 = np.ascontiguousarray(np.asarray(trans), dtype=np.float16)
    transT16 = np.ascontiguousarray(np.asarray(trans).T, dtype=np.float16)
    attn = np.asarray(attn, dtype=np.float16)
    a12 = np.ascontiguousarray(np.concatenate([attn[:D], attn[D:]], axis=1))

    nc = build_nc()
    # identical SPMD program on every core: roll x rows / mask columns by
    # -c*R so each core's own rows are always j-chunks 0..7 (a column
    # permutation inside the softmax sum; the result is unchanged)
    in_maps = [
        {
            "mask16_c": np.ascontiguousarray(
                np.roll(mask16[c * R : (c + 1) * R], -c * R, axis=1)
            ),
            "x_h": np.ascontiguousarray(np.roll(x16, -c * R, axis=0)),
            "trans_h": trans16,
            "transT_h": transT16,
            "a12h": a12,
        }
        for c in range(N_CORES)
    ]
    res = run_bass_kernel_spmd(nc, in_maps, list(range(N_CORES)), trace=_trace)
    global LAST_RESULTS
    LAST_RESULTS = res
    out = np.concatenate(
        [res.results[c]["outT"].T for c in range(N_CORES)], axis=0
    )
    return np.ascontiguousarray(out, dtype=np.float32)


if __name__ == "__main__":
    nc = build_nc()
    print("built OK")


# revision 19
# speedup vs baseline: 1.1868x; 1.0506x over previous
"""Trainium2 Bass kernel for nn_NodeAttentionPerMetaPath (GAT-style node attention).

Reference computation (N=8192, F_IN=256, d=64):
    h      = x @ trans                      # [N, d]
    e1     = h @ attn[:d];  e2 = h @ attn[d:]
    scores = leaky_relu(e1 + e2.T, 0.2)     # [N, N]
    masked = where(mask==0, -1e15, scores)
    out    = softmax(masked, axis=1) @ h    # [N, d]

Sharding: rows r across 8 cores (1024 each); every core computes the full
h/e locally from a replicated fp16 x (no collectives at all).

Algebra (exp monotone, alpha<1):
    exp(leaky(e1+e2)) / exp(a*e1) = B2[j] * max(C[r]*D[j], 1)
    C = exp(.8 e1), D = exp(.8 e2), B2 = exp(.2 e2), C*D*B2 = C*exp(e2)
    out = (P @ h) / (P @ 1),  P = mask * B2 * max(CD, 1)
        = mask * max(C_rep * db, b2)        (one fused 4x tensor_scalar)

Layout: scores live TRANSPOSED [j-part, r-free] from the start.  The mask
is packed to fp16 on the host and transpose-loaded straight from DRAM by
the DMA XBAR (dma_start(transpose=True)), so the [N,N] work needs NO PE
transposes and NO PSUM->SBUF copies: per 128-j chunk it is one TS, one
masked multiply (split DVE/GPSIMD), and one accumulated matmul whose
ones-column yields the softmax denominator for free.
"""

from contextlib import ExitStack

import numpy as np

import concourse.bass as bass
import concourse.bacc as bacc
import concourse.mybir as mybir
import concourse.tile as tile
from concourse.bass_utils import run_bass_kernel_spmd
from concourse.masks import make_identity

f32 = mybir.dt.float32
fp16 = mybir.dt.float16

Exp = mybir.ActivationFunctionType.Exp

N_CORES = 8
N = 8192
F_IN = 256
D = 64
ALPHA = 0.2

R = N // N_CORES   # rows per core
JC = N // 128      # j-chunks
NG = 8             # mask xbar groups
CG = JC // NG      # chunks per group (8)
HQ = 16            # h-compute groups (4 chunks each)

# masked-multiply split per chunk-PAIR [128, 2048]:
# columns [0:TT_SPLIT] on DVE, rest on GPSIMD
TT_SPLIT = 1216


def build_kernel(ctx: ExitStack, tc: tile.TileContext, mask16_c, x_h, trans_h, transT_h, a12h, outT):
    nc = tc.nc

    singles = ctx.enter_context(tc.tile_pool(name="singles", bufs=1))
    hps_pool = ctx.enter_context(tc.tile_pool(name="hps", bufs=2, space="PSUM"))
    ps_o = ctx.enter_context(tc.tile_pool(name="ps_o", bufs=1, space="PSUM"))
    ps_r = ctx.enter_context(tc.tile_pool(name="ps_r", bufs=1, space="PSUM"))
    work = ctx.enter_context(tc.tile_pool(name="work", bufs=3))
    outp = ctx.enter_context(tc.tile_pool(name="outp", bufs=2))
    dram = ctx.enter_context(tc.tile_pool(name="dram", bufs=1, space="DRAM"))

    # ---- DMA streams.  ALL XBAR transposes share one HWDGE queue (sync):
    # concurrent XBAR streams on two queues corrupt each other.
    # x first (it unblocks all compute), then the mask.
    trans_w = singles.tile([128, 2, D + 2], fp16)
    nc.scalar.dma_start(
        out=trans_w[:, :, 0:D], in_=trans_h.rearrange("(c p) d -> p c d", p=128)
    )
    transT_sb = singles.tile([D, F_IN], fp16)
    nc.scalar.dma_start(out=transT_sb, in_=transT_h[:, :])
    a12_sb = singles.tile([D, 2], fp16)
    nc.scalar.dma_start(out=a12_sb, in_=a12h[:, :])
    # xT[p, fc, j] = x[j, fc*128+p]; two tiles so each XBAR dest is contiguous
    xT = [singles.tile([128, 2, N // 2], fp16, tag=f"xT{i}", name=f"xT{i}") for i in range(2)]
    for i in range(2):
        nc.sync.dma_start(
            out=xT[i], in_=x_h[i * (N // 2) : (i + 1) * (N // 2), :], transpose=True
        )

    # mask transpose-stream: mt_g[p, k, r] = mask[r, (g*CG+k)*128+p]
    # rotating buffer pool: the XBAR stream self-throttles on consumption
    mtp = ctx.enter_context(tc.tile_pool(name="mtp", bufs=5))
    mts = []
    for g in range(NG):
        mt = mtp.tile([128, CG, R], fp16, tag="mt", name=f"mt{g}")
        cols = slice(g * CG * 128, (g + 1) * CG * 128)
        nc.sync.dma_start(out=mt, in_=mask16_c[:, cols], transpose=True)
        mts.append(mt)

    ident = singles.tile([128, 128], fp16)
    make_identity(nc, ident)
    ones_row_f = singles.tile([1, D], f32)
    nc.vector.memset(ones_row_f, 1.0)

    # ---- w12 = trans.T @ a12  -> moving-operand columns 64:66
    for fc in range(2):
        wps = hps_pool.tile([128, 2], f32, tag="wps", bufs=1)
        nc.tensor.matmul(
            wps, transT_sb[:, fc * 128 : (fc + 1) * 128], a12_sb, start=True, stop=True
        )
        nc.vector.tensor_copy(trans_w[:, fc, D : D + 2], wps)

    # ---- full h/e per 4-chunk groups.  Inputs are rolled per core so the
    # own 1024 rows are always chunks 0..7 (groups 0 and 1).
    h_sb = singles.tile([128, JC, D + 1], fp16)   # [j%128, jc, h | 1]
    nc.vector.memset(h_sb[:, :, D], 1.0)
    c_all = singles.tile([128, JC], fp16)   # exp(.8 e1)
    db_all = singles.tile([128, JC], f32)  # exp(e2)
    b2_all = singles.tile([128, JC], f32)  # exp(.2 e2)
    C_rep = singles.tile([128, R], fp16)

    for q in range(HQ):
        hps = hps_pool.tile([128, 4, D + 2], f32, tag="hps")
        for k in range(4):
            jc = q * 4 + k
            xa = xT[jc // 32]
            j0 = (jc % 32) * 128
            for fc in range(2):
                nc.tensor.matmul(
                    hps[:, k, :],
                    xa[:, fc, j0 : j0 + 128],
                    trans_w[:, fc, :],
                    start=(fc == 0),
                    stop=(fc == 1),
                )
        cols = slice(q * 4, q * 4 + 4)
        nc.scalar.copy(h_sb[:, cols, 0:D], hps[:, :, 0:D])
        nc.scalar.activation(c_all[:, cols], hps[:, :, D], Exp, scale=1.0 - ALPHA)
        nc.scalar.activation(db_all[:, cols], hps[:, :, D + 1], Exp, scale=1.0)
        nc.scalar.activation(b2_all[:, cols], hps[:, :, D + 1], Exp, scale=ALPHA)

        if q == 1:
            # ---- C_rep: own-row C values replicated across partitions.
            # Transpose each own column of c_all to a [1, 128] row (PE),
            # assemble [1, R], then broadcast via K=1 matmuls.
            # (No DRAM roundtrip, no DMA in the middle of the XBAR stream.)
            c_row1 = singles.tile([1, R], fp16)
            crps = hps_pool.tile([1, 8, 128], fp16, tag="crps", bufs=1)
            for rb in range(8):
                nc.tensor.transpose(crps[:, rb, :], c_all[:, rb : rb + 1], ident)
            nc.vector.tensor_copy(c_row1, crps.rearrange("p a b -> p (a b)"))
            ones_col = singles.tile([1, 128], fp16)
            nc.vector.memset(ones_col, 1.0)
            for half in range(2):
                crp = hps_pool.tile([128, 512], f32, tag="crp", bufs=1)
                nc.tensor.matmul(
                    crp,
                    ones_col,
                    c_row1[:, half * 512 : (half + 1) * 512],
                    start=True,
                    stop=True,
                )
                nc.vector.tensor_copy(C_rep[:, half * 512 : (half + 1) * 512], crp)

    # ---- main loop: chunk-PAIRS.  Two 4x TS fills, one DVE TT + one
    # GPSIMD TT over the flattened [128, 2048], four accumulated matmuls.
    po = [ps_o.tile([D + 1, 512], f32, tag=f"po{i}", name=f"po{i}") for i in range(2)]
    M = mybir.AluOpType.mult
    for p in range(JC // 2):
        jc0 = 2 * p
        mt = mts[jc0 // CG]
        k = jc0 % CG
        vp = work.tile([128, 2, R], fp16, tag="vp", bufs=6)
        for half in range(2):
            jc = jc0 + half
            # v = max(C_rep * exp(e2[jc]), exp(.2 e2[jc]))  (= B2 * max(CD, 1))
            nc.vector.tensor_scalar(
                vp[:, half, :],
                C_rep,
                db_all[:, jc : jc + 1],
                b2_all[:, jc : jc + 1],
                M,
                mybir.AluOpType.max,
            )
        # P = mask * v -> pf (out of place: decouples the TS fill from the
        # matmuls' buffer release), split DVE / GPSIMD
        pf = work.tile([128, 2, R], fp16, tag="pf", bufs=6)
        vf = vp.rearrange("p a b -> p (a b)")
        pff = pf.rearrange("p a b -> p (a b)")
        mf = mt[:, k : k + 2, :].rearrange("p a b -> p (a b)")
        nc.vector.tensor_tensor(pff[:, 0:TT_SPLIT], vf[:, 0:TT_SPLIT], mf[:, 0:TT_SPLIT], M)
        nc.gpsimd.tensor_tensor(pff[:, TT_SPLIT:], vf[:, TT_SPLIT:], mf[:, TT_SPLIT:], M)
        for half in range(2):
            jc = jc0 + half
            for i in range(2):
                nc.tensor.matmul(
                    po[i],
                    h_sb[:, jc, :],
                    pf[:, half, i * 512 : (i + 1) * 512],
                    start=(jc == 0),
                    stop=(jc == JC - 1),
                )

    # ---- normalize: out = numer * (1/denom).  Broadcast the denominator
    # row via a K=1 outer product FIRST, then reciprocal on 64 lanes.
    for i in range(2):
        dcp = outp.tile([1, 512], f32, tag="dcp")
        nc.vector.tensor_copy(dcp, po[i][D : D + 1, :])
        rr = ps_r.tile([D, 512], f32, tag="rr")
        nc.tensor.matmul(rr, ones_row_f, dcp, start=True, stop=True)
        rr_sb = outp.tile([D, 512], f32, tag="rr_sb")
        nc.vector.reciprocal(rr_sb, rr)
        o_t = outp.tile([D, 512], f32, tag="o_t")
        nc.vector.tensor_tensor(o_t, po[i][0:D, :], rr_sb, M)
        nc.gpsimd.dma_start(out=outT[:, i * 512 : (i + 1) * 512], in_=o_t)


def build_nc():
    nc = bacc.Bacc("TRN2", num_devices=N_CORES)
    mask16_c = nc.dram_tensor("mask16_c", [R, N], fp16, kind="ExternalInput")
    x_h = nc.dram_tensor("x_h", [N, F_IN], fp16, kind="ExternalInput")
    trans_h = nc.dram_tensor("trans_h", [F_IN, D], fp16, kind="ExternalInput")
    transT_h = nc.dram_tensor("transT_h", [D, F_IN], fp16, kind="ExternalInput")
    a12h = nc.dram_tensor("a12h", [D, 2], fp16, kind="ExternalInput")
    outT = nc.dram_tensor("outT", [D, R], f32, kind="ExternalOutput")
    with ExitStack() as ctx:
        tc = ctx.enter_context(tile.TileContext(nc))
        build_kernel(
            ctx, tc, mask16_c[:, :], x_h[:, :], trans_h[:, :],
            transT_h[:, :], a12h[:, :], outT[:, :],
        )
    nc.compile()
    return nc


LAST_RESULTS = None


def kernel(x, mask, trans, attn, _trace=False):
    x16 = np.ascontiguousarray(np.asarray(x), dtype=np.float16)
    mask16 = np.ascontiguousarray(np.asarray(mask), dtype=np.float16)
    trans16 = np.ascontiguousarray(np.asarray(trans), dtype=np.float16)
    transT16 = np.ascontiguousarray(np.asarray(trans).T, dtype=np.float16)
    attn = np.asarray(attn, dtype=np.float16)
    a12 = np.ascontiguousarray(np.concatenate([attn[:D], attn[D:]], axis=1))

    nc = build_nc()
    # identical SPMD program on every core: roll x rows / mask columns by
    # -c*R so each core's own rows are always j-chunks 0..7 (a column
    # permutation inside the softmax sum; the result is unchanged)
    in_maps = [
        {
            "mask16_c": np.ascontiguousarray(
                np.roll(mask16[c * R : (c + 1) * R], -c * R, axis=1)
            ),
            "x_h": np.ascontiguousarray(np.roll(x16, -c * R, axis=0)),
            "trans_h": trans16,
            "transT_h": transT16,
            "a12h": a12,
        }
        for c in range(N_CORES)
    ]
    res = run_bass_kernel_spmd(nc, in_maps, list(range(N_CORES)), trace=_trace)
    global LAST_RESULTS
    LAST_RESULTS = res
    out = np.concatenate(
        [res.results[c]["outT"].T for c in range(N_CORES)], axis=0
    )
    return np.ascontiguousarray(out, dtype=np.float32)


if __name__ == "__main__":
    nc = build_nc()
    print("built OK")
